# revision 18
# baseline (speedup 1.0000x reference)
"""Trainium2 Bass kernel for the reference MultiHeadAttention module.

Problem: B=32, T=512, D=1024, H=16, HD=64 (see reference semantics below).

Reference computation (unusual orientation: keys index rows, queries index
softmax axis, no 1/sqrt(d) scale):
    h  = x @ Wi + bi
    k/q/v = per-head h @ W{k,q,v}[h] + b (head-stacked weights)
    wei[b,h,t,s] = k[b,h,t,:] . q[b,h,s,:]      (t = key idx, s = query idx)
    wei masked to s <= t, softmax over s
    out = (wei @ v) concat-heads @ Wo + bo

Key algebraic simplification: h is used ONLY to form q/k/v, so Wi folds into
the projections host-side:  q = x @ (Wi Wq) + (bi Wq + bq)  etc.  This removes
the entire in_proj matmul chain on device (1/5 of the dense FLOPs).

Sharding: data-parallel over batch. Each of the 8 cores processes 4 batches
(2048 tokens) with replicated weights; no collectives.

Per-core dataflow (everything SBUF-resident, zero DRAM spills):
  Phase A: three projection chains from xT (fp16, feature-major):
    q^T, k^T feature-major (fp16) via stationary Wq'/Wk' slices (fp16, FWL),
    4 moving 512-token slices per stationary; repeated-stationary matmuls
    set ldweights=False (validated bit-exact on HW) to elide LDWEIGHTS;
    v token-major (bf16, [v|1] per head) via stationary xT slices.
    Weight sets rotate through one 2-deep tile pool: wq -> wk -> wv -> wo.
  Phase B, interleaved at head-pair / out_proj-group granularity so the PE
  always has runnable out_proj matmuls while exp/normalize resolves:
    S^T[s,t] per head from resident q/k slices (fp16, base-partition 0/64
    pairs -> concurrent row-group matmuls on HW); the pair shares one
    [128,1024] 2-bank PSUM tile so exp runs once per (pair, i-tile);
    causal mask added via a tiny accumulated matmul (triT^T @ I = -30000
    above the diagonal) -- no vector-engine masking work at all,
    P^T = exp(S^T) on ACT -> bf16 (no max-subtraction: |logits| <= ~55 is
    safe in fp32 since the reference softmax ratio cancels the shift),
    O^T = [V | 1]^T @ P^T per head (ones column = softmax denominator),
    reciprocal on DVE, partition_broadcast on Pool, multiply -> oT fp16,
    out_proj from oT slices (fp16 stationary, FWL) -> fp16 output.
  Host converts fp16 -> f32 and adds the folded constant (bv' Wo + bo).

  fp8e4m3 (DoubleRow) was tried for out_proj and the v projection: both
  fail the 2e-2 gate (~3.4% rel; summation error carries the per-element
  fp8 sigma with no averaging benefit), hence the FP8_* flags stay off.

Biases are folded host-side: bi through the fused projections; bq/bk applied
on-device only if nonzero (zero in setup_inputs); bv'Wo + bo added on host.
"""

import sys

sys.path.insert(0, "/opt/trn_rl_repo")

import numpy as np

import concourse.bacc as bacc
import concourse.mybir as mybir
from concourse import bass_utils
from concourse.tile import TileContext

F32 = mybir.dt.float32
FP8 = mybir.dt.float8e4
F16 = mybir.dt.float16
BF16 = mybir.dt.bfloat16
AF = mybir.ActivationFunctionType

B, T, D, H, HD = 32, 512, 1024, 16, 64
NCORES = 8
BN = B // NCORES          # batches per core = 4
TOK = BN * T              # tokens per core = 2048
NKT = D // 128            # 8 contraction tiles
MASK_NEG = -30000.0       # fp16-safe; exp(-30000 + 55) == 0 in fp32
SKIP_LDW = True           # elide LDWEIGHTS when the stationary is unchanged
FP8_OUT = False           # fp8 out_proj fails the 2e-2 gate (rel 0.034)
FP8_V = False             # fp8 v-projection also fails the 2e-2 gate (rel 0.034)
WO_SCALE = 64.0

_CACHE = {}


def _build(with_qk_bias: bool, repeat: int = 1):
    nc = bacc.Bacc("TRN2", target_bir_lowering=False, debug=False,
                   num_devices=NCORES)

    xT = nc.dram_tensor("xT", [D, TOK], F16, kind="ExternalInput")
    wq = nc.dram_tensor("wq", [D, D], F16, kind="ExternalInput")
    wk = nc.dram_tensor("wk", [D, D], F16, kind="ExternalInput")
    if FP8_V:
        x8 = nc.dram_tensor("x8", [D // 2, 2 * TOK], FP8, kind="ExternalInput")
        wv = nc.dram_tensor("wv", [D // 2, 2 * D], FP8, kind="ExternalInput")
    else:
        wv = nc.dram_tensor("wv", [D, D], F16, kind="ExternalInput")
    if FP8_OUT:
        wo = nc.dram_tensor("wo", [D // 2, 2 * D], FP8, kind="ExternalInput")
    else:
        wo = nc.dram_tensor("wo", [D, D], F16, kind="ExternalInput")
    triT = nc.dram_tensor("triT", [128, 128], F16, kind="ExternalInput")
    iden = nc.dram_tensor("iden", [128, 128], F16, kind="ExternalInput")
    onesc = nc.dram_tensor("onesc", [128, H], BF16, kind="ExternalInput")
    if with_qk_bias:
        bq2 = nc.dram_tensor("bq2", [128, NKT], F32, kind="ExternalInput")
        bk2 = nc.dram_tensor("bk2", [128, NKT], F32, kind="ExternalInput")
    out = nc.dram_tensor("out", [TOK, D], F16, kind="ExternalOutput")

    with TileContext(nc) as tc:
      for _rep in range(repeat):
        with tc.tile_pool(name="res", bufs=1) as rpool, \
             tc.tile_pool(name="w", bufs=1) as wpool:
            tri_sb = rpool.tile([128, 128], F16, tag="tri", name="tri")
            nc.sync.dma_start(tri_sb[:], triT[:])
            id_sb = rpool.tile([128, 128], F16, tag="iden", name="iden")
            nc.sync.dma_start(id_sb[:], iden[:])
            ones16_sb = rpool.tile([128, H], BF16, tag="ones16", name="ones16")
            nc.sync.dma_start(ones16_sb[:], onesc[:])
            if with_qk_bias:
                bq_sb = rpool.tile([128, NKT], F32, tag="bq", name="bq")
                bk_sb = rpool.tile([128, NKT], F32, tag="bk", name="bk")
                nc.sync.dma_start(bq_sb[:], bq2[:])
                nc.sync.dma_start(bk_sb[:], bk2[:])

            # resident activation tensors (outer pool: survive into phase B)
            q_res = [rpool.tile([128, TOK], F16, tag=f"q{e}", name=f"q{e}")
                     for e in range(NKT)]
            k_res = [rpool.tile([128, TOK], F16, tag=f"k{e}", name=f"k{e}")
                     for e in range(NKT)]
            v_res = [rpool.tile([128, H * 65], BF16, tag=f"v{i}", name=f"v{i}")
                     for i in range(TOK // 128)]

            # ---------------- Phase A: fused projections --------------
            with tc.tile_pool(name="actA", bufs=1) as apool, \
                 tc.tile_pool(name="psA", bufs=1, space="PSUM") as pspool:
                # preload the Exp table so phase B's first exp is not gated
                # on LoadActFuncSet
                warm = apool.tile([1, 2], F32, tag="warm", name="warm")
                with nc.allow_low_precision(reason="act warmup"):
                    nc.scalar.activation(warm[:], ones16_sb[0:1, 0:2], AF.Exp)
                xc = [apool.tile([128, TOK], F16, tag=f"xc{k}", name=f"xc{k}")
                      for k in range(NKT)]
                wq_sb = [wpool.tile([128, D], F16, tag=f"w{k}", bufs=2,
                                    name=f"wq{k}") for k in range(NKT)]
                for k in range(NKT):
                    nc.sync.dma_start(wq_sb[k][:], wq[128 * k:128 * (k + 1), :])
                    nc.sync.dma_start(xc[k][:, 0:1024],
                                      xT[128 * k:128 * (k + 1), 0:1024])
                for k in range(NKT):
                    nc.sync.dma_start(xc[k][:, 1024:TOK],
                                      xT[128 * k:128 * (k + 1), 1024:TOK])
                wk_sb = [wpool.tile([128, D], F16, tag=f"w{k}", bufs=2,
                                    name=f"wk{k}") for k in range(NKT)]
                for k in range(NKT):
                    nc.sync.dma_start(wk_sb[k][:], wk[128 * k:128 * (k + 1), :])

                def proj_chain(w_sb, dst, b_ap):
                    """dst[e][:, :] = (w-chain).T @ x-chain, feature-major."""
                    for e in range(NKT):
                        ps = [pspool.tile([128, 512], F32, tag=f"psA{t}",
                                          bufs=2, name=f"psA{t}")
                              for t in range(4)]
                        for k in range(NKT):
                            for t in range(4):
                                inst = nc.tensor.matmul(
                                    ps[t][:],
                                    w_sb[k][:, 128 * e:128 * (e + 1)],
                                    xc[k][:, 512 * t:512 * (t + 1)],
                                    start=(k == 0), stop=(k == NKT - 1),
                                    skip_group_check=True)
                                if SKIP_LDW and t > 0:
                                    inst.ins.ldweights = False
                        for t in range(4):
                            d_ap = dst[e][:, 512 * t:512 * (t + 1)]
                            with nc.allow_low_precision(reason="fp16 qk store"):
                                if with_qk_bias:
                                    bias = (bq_sb if b_ap == "bq"
                                            else bk_sb)[:, e:e + 1]
                                    nc.vector.tensor_scalar_add(
                                        d_ap, ps[t][:], bias)
                                else:
                                    nc.vector.tensor_copy(d_ap, ps[t][:])

                proj_chain(wq_sb, q_res, "bq")
                if FP8_V:
                    wv_sb = [wpool.tile([128, 2 * D], FP8, tag=f"w{2 * k}",
                                        bufs=2, name=f"wv{k}")
                             for k in range(NKT // 2)]
                    x8_sb = [wpool.tile([128, 2 * TOK], FP8, tag=f"x8_{k}",
                                        bufs=1, name=f"x8_{k}")
                             for k in range(NKT // 2)]
                    for k in range(NKT // 2):
                        nc.sync.dma_start(wv_sb[k][:],
                                          wv[128 * k:128 * (k + 1), :])
                        nc.sync.dma_start(x8_sb[k][:],
                                          x8[128 * k:128 * (k + 1), :])
                else:
                    wv_sb = [wpool.tile([128, D], F16, tag=f"w{k}", bufs=2,
                                        name=f"wv{k}") for k in range(NKT)]
                    for k in range(NKT):
                        nc.sync.dma_start(wv_sb[k][:],
                                          wv[128 * k:128 * (k + 1), :])
                proj_chain(wk_sb, k_res, "bk")
                if FP8_OUT:
                    wo_sb = [wpool.tile([128, 2 * D], FP8, tag=f"w{2 * k}",
                                        bufs=2, name=f"wo{k}")
                             for k in range(NKT // 2)]
                    for k in range(NKT // 2):
                        nc.sync.dma_start(
                            wo_sb[k][:], wo[128 * k:128 * (k + 1), :])
                else:
                    wo_sb = [wpool.tile([128, D], F16, tag=f"w{k}", bufs=2,
                                        name=f"wo{k}") for k in range(NKT)]
                    for k in range(NKT):
                        nc.sync.dma_start(wo_sb[k][:],
                                          wo[128 * k:128 * (k + 1), :])

                # v chain: token-major, stationary = xT slices
                for tt in range(TOK // 128):
                    pv = [pspool.tile([128, 512], F32, tag=f"psA{nn}",
                                      bufs=2, name=f"psV{nn}")
                          for nn in range(2)]
                    if FP8_V:
                        for k in range(NKT // 2):
                            lhsT = x8_sb[k][:].rearrange(
                                "p (j t) -> p j t",
                                t=TOK)[:, :, 128 * tt:128 * (tt + 1)]
                            rhs3 = wv_sb[k][:].rearrange(
                                "p (j n) -> p j n", n=D)
                            for nn in range(2):
                                inst = nc.tensor.matmul(
                                    pv[nn][:], lhsT,
                                    rhs3[:, :, 512 * nn:512 * (nn + 1)],
                                    start=(k == 0), stop=(k == NKT // 2 - 1),
                                    perf_mode=mybir.MatmulPerfMode.DoubleRow,
                                    skip_group_check=True)
                                if SKIP_LDW and nn > 0:
                                    inst.ins.ldweights = False
                    else:
                        for k in range(NKT):
                            for nn in range(2):
                                inst = nc.tensor.matmul(
                                    pv[nn][:],
                                    xc[k][:, 128 * tt:128 * (tt + 1)],
                                    wv_sb[k][:, 512 * nn:512 * (nn + 1)],
                                    start=(k == 0), stop=(k == NKT - 1),
                                    skip_group_check=True)
                                if SKIP_LDW and nn > 0:
                                    inst.ins.ldweights = False
                    v3 = v_res[tt][:].rearrange("p (h e) -> p h e", e=65)
                    with nc.allow_low_precision(reason="bf16 v store"):
                        for nn in range(2):
                            if FP8_V:
                                nc.scalar.activation(
                                    v3[:, 8 * nn:8 * (nn + 1), 0:64],
                                    pv[nn][:].rearrange("p (h e) -> p h e",
                                                        e=64),
                                    AF.Copy, scale=1.0 / WO_SCALE)
                            else:
                                nc.vector.tensor_copy(
                                    v3[:, 8 * nn:8 * (nn + 1), 0:64],
                                    pv[nn][:].rearrange("p (h e) -> p h e",
                                                        e=64))
                    nc.vector.tensor_copy(v3[:, :, 64], ones16_sb[:])

            # ---------------- Phase B: attention + out_proj, interleaved
            # at head-pair / out_proj-group granularity so the PE always has
            # runnable out_proj matmuls while exp/normalize of the current
            # pair resolves on ACT/DVE/Pool.
            with tc.tile_pool(name="actB", bufs=1) as apool, \
                 tc.tile_pool(name="psB", bufs=1, space="PSUM") as psB:

                def pair_unit(b, m, mid_cb=None):
                    """Head-pair m of batch b -> oT[m] tile (new generation).

                    The pair's two heads share one [128,1024] 2-bank PSUM tile
                    (head j%2 in columns 512*(j%2)+), so exp runs once per
                    (pair, i) over a strided 2-range AP instead of twice."""
                    r0 = 512 * b
                    if FP8_OUT:
                        if m % 2 == 0:
                            pair_unit.oT8[m // 2] = apool.tile(
                                [128, 1024], FP8, tag=f"oT8_{m // 2}", bufs=2,
                                name=f"oT8_{m // 2}")
                        full = pair_unit.oT8[m // 2]
                        oT_m = full[:, 512 * (m % 2):512 * (m % 2) + 512]
                        ret = full if m % 2 == 1 else None
                    else:
                        oT_m = apool.tile([128, 512], F16, tag=f"oT{m}", bufs=2,
                                          name=f"oT{m}")
                        ret = oT_m
                    pos = {}
                    pts = {}
                    for i in range(4):
                        w0 = 128 * i
                        ps = psB.tile([128, 1024], F32, tag="ps",
                                      bufs=3, name=f"ps{i}")
                        for j in (2 * m, 2 * m + 1):
                            off = 64 * (j % 2)
                            c0 = 512 * (j % 2)
                            nc.tensor.matmul(
                                ps[:, c0 + w0:c0 + 512],
                                q_res[m][off:off + 64, r0 + w0:r0 + w0 + 128],
                                k_res[m][off:off + 64, r0 + w0:r0 + 512],
                                start=True, stop=False,
                                skip_group_check=True)
                        for j in (2 * m, 2 * m + 1):
                            # additive causal mask on the diagonal block
                            c0 = 512 * (j % 2)
                            inst = nc.tensor.matmul(
                                ps[:, c0 + w0:c0 + w0 + 128], tri_sb[:],
                                id_sb[:], start=False, stop=True,
                                skip_group_check=True)
                            if SKIP_LDW and j % 2 == 1:
                                inst.ins.ldweights = False
                        pt = apool.tile([128, 1024], BF16, tag="pt",
                                        bufs=5, name=f"pt{i}")
                        ps3 = ps[:].rearrange("p (j c) -> p j c", c=512)
                        pt3 = pt[:].rearrange("p (j c) -> p j c", c=512)
                        with nc.allow_low_precision(reason="bf16 exp"):
                            nc.scalar.activation(
                                pt3[:, :, w0:512], ps3[:, :, w0:512], AF.Exp)
                        pts[i] = pt
                    if mid_cb is not None:
                        mid_cb()
                    for j in (2 * m, 2 * m + 1):
                        po = psB.tile([65, 512], F32, tag="po", bufs=2,
                                      name=f"po{j % 2}")
                        pos[j] = po
                        c0 = 512 * (j % 2)
                        for i in range(4):
                            w0 = 128 * i
                            v3 = v_res[4 * b + i][:].rearrange(
                                "p (h e) -> p h e", e=65)
                            nc.tensor.matmul(
                                po[0:65, w0:512],
                                v3[:, j, :],
                                pts[i][:, c0 + w0:c0 + 512],
                                start=(i == 0), stop=(i == 3),
                                skip_group_check=True)
                    for j in (2 * m, 2 * m + 1):
                        off = 64 * (j % 2)
                        rs = apool.tile([1, 512], F32, tag="rs", bufs=4,
                                        name="rs")
                        with nc.allow_low_precision(reason="recip"):
                            nc.vector.reciprocal(rs[:], pos[j][64:65, :])
                        rb = apool.tile([64, 512], F32, tag="rb", bufs=4,
                                        name="rb")
                        nc.gpsimd.partition_broadcast(rb[:], rs[:])
                        with nc.allow_low_precision(reason="fp16 oT"):
                            nc.vector.tensor_mul(
                                oT_m[off:off + 64, :],
                                pos[j][0:64, :], rb[:])
                    return ret
                pair_unit.oT8 = [None] * 4

                def proj_unit(b, oT, tt):
                    """out_proj token-rows [128*tt, 128*tt+128) of batch b."""
                    r0 = 512 * b
                    pf = psB.tile([128, 1024], F32, tag="ps", bufs=3,
                                  name="pf")
                    if FP8_OUT:
                        for k in range(NKT // 2):
                            lhsT = oT[k][:].rearrange(
                                "p (j t) -> p j t", t=512)[:, :, 128 * tt:128 * (tt + 1)]
                            rhs3 = wo_sb[k][:].rearrange(
                                "p (j n) -> p j n", n=D)
                            for nn in range(2):
                                inst = nc.tensor.matmul(
                                    pf[:, 512 * nn:512 * (nn + 1)],
                                    lhsT, rhs3[:, :, 512 * nn:512 * (nn + 1)],
                                    start=(k == 0), stop=(k == NKT // 2 - 1),
                                    perf_mode=mybir.MatmulPerfMode.DoubleRow,
                                    skip_group_check=True)
                                if SKIP_LDW and nn > 0:
                                    inst.ins.ldweights = False
                    else:
                        for k in range(NKT):
                            for nn in range(2):
                                inst = nc.tensor.matmul(
                                    pf[:, 512 * nn:512 * (nn + 1)],
                                    oT[k][:, 128 * tt:128 * (tt + 1)],
                                    wo_sb[k][:, 512 * nn:512 * (nn + 1)],
                                    start=(k == 0), stop=(k == NKT - 1),
                                    skip_group_check=True)
                                if SKIP_LDW and nn > 0:
                                    inst.ins.ldweights = False
                    os_ = apool.tile([128, 1024], F16, tag="os", bufs=3,
                                     name=f"os{tt}")
                    with nc.allow_low_precision(reason="fp16 out"):
                        if FP8_OUT:
                            nc.scalar.activation(os_[:], pf[:], AF.Copy,
                                                 scale=1.0 / WO_SCALE)
                        else:
                            nc.vector.tensor_copy(os_[:], pf[:])
                    nc.sync.dma_start(
                        out[r0 + 128 * tt:r0 + 128 * (tt + 1), :], os_[:])

                oT_prev = [t for m in range(H // 2)
                           if (t := pair_unit(0, m)) is not None]
                for b in range(1, BN):
                    oT_cur = []
                    for m in range(H // 2):
                        t = pair_unit(b, m)
                        if t is not None:
                            oT_cur.append(t)
                        if m % 2 == 1:
                            proj_unit(b - 1, oT_prev, m // 2)
                    oT_prev = oT_cur
                for tt in range(4):
                    proj_unit(BN - 1, oT_prev, tt)

    nc.compile()
    return nc


def _ensure_built(with_qk_bias: bool, repeat: int = 1):
    key = (with_qk_bias, repeat)
    if key not in _CACHE:
        _CACHE[key] = _build(with_qk_bias, repeat)
    return _CACHE[key]


def _prepare(x, Wi, bi, Wk, bk, Wq, bq, Wv, bv, Wo, bo):
    """Host-side prep: returns (in_maps, out_const, with_qk_bias)."""
    x = np.asarray(x, np.float32)
    Wi, bi = np.asarray(Wi, np.float64), np.asarray(bi, np.float64)
    Wk, bk = np.asarray(Wk, np.float64), np.asarray(bk, np.float64)
    Wq, bq = np.asarray(Wq, np.float64), np.asarray(bq, np.float64)
    Wv, bv = np.asarray(Wv, np.float64), np.asarray(bv, np.float64)
    Wo, bo = np.asarray(Wo, np.float64), np.asarray(bo, np.float64)

    # flatten head-stacked weights: col f = h*HD + e, then fold Wi through
    wq_f = Wq.transpose(1, 0, 2).reshape(D, D)
    wk_f = Wk.transpose(1, 0, 2).reshape(D, D)
    wv_f = Wv.transpose(1, 0, 2).reshape(D, D)
    wq_fold = (Wi @ wq_f).astype(np.float16)
    wk_fold = (Wi @ wk_f).astype(np.float16)
    wv_fold = (Wi @ wv_f).astype(np.float16)
    bq_fold = (bi @ wq_f + bq.reshape(-1)).astype(np.float32)
    bk_fold = (bi @ wk_f + bk.reshape(-1)).astype(np.float32)
    bv_fold = (bi @ wv_f + bv.reshape(-1))
    out_const = (bv_fold @ Wo + bo).astype(np.float32)  # added host-side

    with_qk_bias = bool(np.any(bq_fold) or np.any(bk_fold))

    # additive causal mask for the diagonal block, pre-transposed so that
    # triT.T @ I applies mask[s, t] = MASK_NEG where s > t
    mask = ((np.triu(np.ones((128, 128))) - 1.0) * -MASK_NEG)  # [s, t]
    triT = np.ascontiguousarray(mask.T).astype(np.float16)
    iden = np.eye(128, dtype=np.float16)
    onesc = np.ones((128, H), np.float32)

    import ml_dtypes
    if FP8_OUT:
        # paired DoubleRow layout: wo8[128*kp + p, 1024*j + n]
        #   = Wo[256*kp + 128*j + p, n] * WO_SCALE
        wo8 = (Wo * WO_SCALE).reshape(4, 2, 128, D).transpose(0, 2, 1, 3)
        wo_host = np.ascontiguousarray(wo8.reshape(D // 2, 2 * D)).astype(
            ml_dtypes.float8_e4m3)
    else:
        wo_host = Wo.astype(np.float16)
    if FP8_V:
        wv8 = (Wi @ wv_f * WO_SCALE).reshape(4, 2, 128, D).transpose(0, 2, 1, 3)
        wv_host = np.ascontiguousarray(wv8.reshape(D // 2, 2 * D)).astype(
            ml_dtypes.float8_e4m3)
    else:
        wv_host = wv_fold
    shared = {"wq": wq_fold, "wk": wk_fold, "wv": wv_host,
              "wo": wo_host, "triT": triT, "iden": iden,
              "onesc": onesc.astype(ml_dtypes.bfloat16)}
    if with_qk_bias:
        shared["bq2"] = np.ascontiguousarray(bq_fold.reshape(NKT, 128).T)
        shared["bk2"] = np.ascontiguousarray(bk_fold.reshape(NKT, 128).T)

    in_maps = []
    for c in range(NCORES):
        xs = x[BN * c:BN * (c + 1)].reshape(TOK, D)
        m = dict(shared)
        xsT = np.ascontiguousarray(xs.T)
        m["xT"] = xsT.astype(np.float16)
        if FP8_V:
            # paired DoubleRow layout: x8[128*kp + p, TOK*j + t]
            #   = xT[256*kp + 128*j + p, t]
            x8v = xsT.reshape(4, 2, 128, TOK).transpose(0, 2, 1, 3)
            m["x8"] = np.ascontiguousarray(
                x8v.reshape(D // 2, 2 * TOK)).astype(ml_dtypes.float8_e4m3)
        in_maps.append(m)
    return in_maps, out_const, with_qk_bias


def kernel(x, Wi, bi, Wk, bk, Wq, bq, Wv, bv, Wo, bo):
    in_maps, out_const, with_qk_bias = _prepare(
        x, Wi, bi, Wk, bk, Wq, bq, Wv, bv, Wo, bo)
    nc = _ensure_built(with_qk_bias)
    res = bass_utils.run_bass_kernel_spmd(nc, in_maps, core_ids=list(range(NCORES)))
    outs = [np.asarray(res.results[c]["out"], np.float32) for c in range(NCORES)]
    full = np.concatenate(outs, axis=0).reshape(B, T, D)
    full += out_const[None, None, :]
    return full
